# revision 4
# baseline (speedup 1.0000x reference)
"""Trainium2 Bass kernel for stacked-Linear dense MLP:
    out[1024, 32768] = x[1024, 512] @ W[32768, 512].T + b[32768]

Strategy: column-parallel over 8 NeuronCores. Core c owns W rows
[c*4096, (c+1)*4096) -> output columns of the same range; x replicated.
On-chip: bf16 matmul (fp32 PSUM accumulate), bias added on DVE during
PSUM->SBUF evacuation (cast to bf16), bf16 output upcast to fp32 on host.

v2 structure (from NTFF trace analysis of v1 @75.7us):
  - Steady-state MM stream was already optimal (216ns/MM); the waste was
    a ~14us dead start (DMA batch sems fire at end of the round-robin
    mix) and a ~9us postamble (per-DMA semaphore drains).
  - Inputs now land as prioritized per-ring batch chains whose sems fire
    in consumption order:
      scalar ring: bias -> x[m0..m2] -> x[m3..m7]
      sync ring:   W[n0] -> W[n1] -> W[n2n3] -> W[n4n5] -> W[n6n7]
    (chained with add_dep_helper: concurrent batches on one ring
    round-robin and ALL complete late; chaining serializes them.)
  - Outputs batched to cut DMA count 64 -> 33: quads (n0-3), pairs
    (n4n5), singles n6/n7 so the final sweep drains in real time; the
    very last tile (n7,m7) is split across both rings.
  - 4 warmup matmuls bridge engine-preamble-end to first-data and start
    the HAM un-throttle window early.
"""

import sys

sys.path.insert(0, "/opt/trn_rl_repo")

import numpy as np
import ml_dtypes

# ---- problem constants (hardcoded per contract) ----
B = 1024          # batch (matmul M)
K = 512           # hidden size (contraction)
N_TOTAL = 32768   # hidden_size * map_element_size
N_CORES = 8
NS = N_TOTAL // N_CORES  # 4096 output cols per core

KT = K // 128     # 4 k-tiles
MT = B // 128     # 8 m-tiles
NCH = NS // 512   # 8 n-chunks of 512 (one PSUM bank each)

W_BATCHES = [[0], [1], [2, 3], [4, 5], [6, 7]]  # n-chunks per sync-ring link
X_BATCHES = [[0, 1, 2], [3, 4, 5, 6, 7]]        # m-tiles per scalar-ring link
WARM_MMS = 4

_CACHE = {}


def _build_program():
    import concourse.bacc as bacc
    import concourse.mybir as mybir
    from concourse.bass import ds, ts
    from concourse.tile import TileContext
    from concourse.tile_rust import add_dep_helper
    from contextlib import ExitStack

    nc = bacc.Bacc("TRN2", target_bir_lowering=False, debug=False)

    out_dt = mybir.dt.bfloat16

    # host-prepared SBUF-image layouts (see _prep_inputs)
    xh = nc.dram_tensor("xh", [128, MT, KT, 128], mybir.dt.bfloat16, kind="ExternalInput").ap()
    wh = nc.dram_tensor("wh", [128, NCH, KT, 512], mybir.dt.bfloat16, kind="ExternalInput").ap()
    bias = nc.dram_tensor("bias", [1, NS], mybir.dt.float32, kind="ExternalInput").ap()
    out = nc.dram_tensor("out", [B, NS], out_dt, kind="ExternalOutput").ap()

    with TileContext(nc) as tc:
        with ExitStack() as ctx:
            const = ctx.enter_context(tc.tile_pool(name="const", bufs=1))
            # out tiles: quads (n0-3), pairs (n4n5), singles (n6, n7)
            oquad = ctx.enter_context(tc.tile_pool(name="oquad", bufs=1))
            opair = ctx.enter_context(tc.tile_pool(name="opair", bufs=1))
            osing = ctx.enter_context(tc.tile_pool(name="osing", bufs=1))
            psum = ctx.enter_context(tc.tile_pool(name="psum", bufs=7, space="PSUM"))
            wpool = ctx.enter_context(tc.tile_pool(name="wpool", bufs=1))

            # --- PE warmup ASAP: gpsimd memset + warmup matmuls start the
            # HAM un-throttle window and bridge to first-data (~10.5us).
            warm = const.tile([128, 512], mybir.dt.bfloat16, tag="warm")
            warm_ps = psum.tile([128, 512], mybir.dt.float32, tag="warmps", bufs=1)
            nc.gpsimd.memset(warm[:], 0)
            for _ in range(WARM_MMS):
                nc.tensor.matmul(
                    warm_ps[:], lhsT=warm[:, 0:128], rhs=warm[:], start=True, stop=True
                )
            warm_sink = const.tile([128, 512], mybir.dt.float32, tag="warmsink")
            nc.vector.tensor_copy(warm_sink[:], warm_ps[:])  # keep warmups live

            # --- scalar ring: bias (tiny, sem fires early in the RR mix),
            # then chained x batches in consumption order.
            bias_sb = const.tile([128, NS], mybir.dt.float32, tag="bias")
            nc.scalar.dma_start(bias_sb[0:1, :], bias)
            xh_sb = const.tile([128, MT, KT, 128], mybir.dt.bfloat16, tag="xh")
            prev = None
            for bi, ms in enumerate(X_BATCHES):
                dma = nc.scalar.dma_start(
                    xh_sb[:, ds(ms[0], len(ms))], xh[:, ds(ms[0], len(ms))]
                )
                if prev is not None:
                    add_dep_helper(dma.ins, prev.ins, reason="chain x DMAs")
                prev = dma

            # --- bias partition-broadcasts (gated on the bias DMA sem)
            for n in range(NCH):
                nc.gpsimd.partition_broadcast(
                    bias_sb[:, ds(n * 512, 512)], bias_sb[0:1, ds(n * 512, 512)]
                )

            # --- sync ring: chained W batches in consumption order
            wt_tiles = []
            n2cl = {}
            prev = None
            for c, ns_ in enumerate(W_BATCHES):
                t = wpool.tile([128, len(ns_), KT, 512], mybir.dt.bfloat16, tag=f"wt{c}")
                dma = nc.sync.dma_start(t[:], wh[:, ds(ns_[0], len(ns_))])
                if prev is not None:
                    add_dep_helper(dma.ins, prev.ins, reason="chain W DMAs")
                prev = dma
                wt_tiles.append(t)
                for i, n in enumerate(ns_):
                    n2cl[n] = (c, i)

            # --- out tile plumbing -------------------------------------
            # n-chunk -> (group kind, base n-chunk, offset within tile)
            def out_slot(n):
                if n < 4:
                    return ("q", 0, n)          # quad tile holds n0..n3
                if n < 6:
                    return ("p", 4, n - 4)      # pair tile holds n4n5
                return ("s", n, 0)              # singles n6, n7

            quad_tiles = {}   # m -> tile [128, 2048]
            pair_tiles = {}   # m -> tile [128, 1024]

            ring = [nc.sync, nc.scalar]
            out_cnt = 0

            # --- main loop: n-chunks outer so PE tracks W arrival
            for n in range(NCH):
                for m in range(MT):
                    c, ln = n2cl[n]
                    ps = psum.tile([128, 512], mybir.dt.float32)
                    for k in range(KT):
                        nc.tensor.matmul(
                            ps[:],
                            lhsT=xh_sb[:, m, k, :],
                            rhs=wt_tiles[c][:, ln, k, :],
                            start=(k == 0),
                            stop=(k == KT - 1),
                        )
                    kind, nb, off = out_slot(n)
                    if kind == "q":
                        if m not in quad_tiles:
                            quad_tiles[m] = oquad.tile([128, 2048], out_dt, tag=f"oq{m}", name=f"oq{m}")
                        ot, width = quad_tiles[m], 2048
                        last_n = nb + 3
                    elif kind == "p":
                        if m not in pair_tiles:
                            pair_tiles[m] = opair.tile([128, 1024], out_dt, tag=f"op{m}", name=f"op{m}")
                        ot, width = pair_tiles[m], 1024
                        last_n = nb + 1
                    else:
                        ot, width = osing.tile([128, 512], out_dt, tag=f"os{n}_{m}", name=f"os{n}_{m}"), 512
                        last_n = nb
                    nc.vector.tensor_add(
                        ot[:, ds(off * 512, 512)], ps[:], bias_sb[:, ds(n * 512, 512)]
                    )
                    if n == last_n:
                        dst = out[ts(m, 128), ds(nb * 512, width)]
                        if n == NCH - 1 and m == MT - 1:
                            # very last tile: split across both rings
                            nc.sync.dma_start(dst[0:64], ot[0:64, :])
                            nc.scalar.dma_start(dst[64:128], ot[64:128, :])
                        else:
                            ring[out_cnt % 2].dma_start(dst, ot[:])
                            out_cnt += 1
                    if kind == "p" and n == nb:
                        pass  # pair not complete yet
            # clear dicts so a rebuild (never happens) wouldn't alias
            quad_tiles.clear()
            pair_tiles.clear()

    nc.compile()
    return nc


def _get_program():
    if "nc" not in _CACHE:
        _CACHE["nc"] = _build_program()
    return _CACHE["nc"]


def _prep_inputs(x, W, b):
    bf16 = ml_dtypes.bfloat16
    x = np.asarray(x, dtype=np.float32)
    W = np.asarray(W, dtype=np.float32)
    b = np.asarray(b, dtype=np.float32)
    # xh[p, mt, kt, m] = x[mt*128 + m, kt*128 + p]
    xh = np.ascontiguousarray(
        x.T.reshape(KT, 128, MT, 128).transpose(1, 2, 0, 3)
    ).astype(bf16)
    in_maps = []
    for c in range(N_CORES):
        sl = slice(c * NS, (c + 1) * NS)
        # wh[p, n, kt, j] = W[c*NS + n*512 + j, kt*128 + p]
        wh = np.ascontiguousarray(
            W[sl, :].T.reshape(KT, 128, NCH, 512).transpose(1, 2, 0, 3)
        ).astype(bf16)
        bc = np.ascontiguousarray(b[sl].reshape(1, NS))
        in_maps.append({"xh": xh, "wh": wh, "bias": bc})
    return in_maps


def _run(x, W, b, trace=False):
    from concourse.bass_utils import run_bass_kernel_spmd

    nc = _get_program()
    in_maps = _prep_inputs(x, W, b)
    res = run_bass_kernel_spmd(nc, in_maps, list(range(N_CORES)), trace=trace)
    _CACHE["last_result"] = res
    out = np.concatenate([r["out"] for r in res.results], axis=1)
    return out.astype(np.float32)


def kernel(x, W, b):
    return _run(x, W, b, trace=False)


def kernel_profiled(x, W, b):
    """Same as kernel() but with NTFF tracing; returns (out, BassKernelResults)."""
    out = _run(x, W, b, trace=True)
    return out, _CACHE["last_result"]


# revision 13
# speedup vs baseline: 1.1742x; 1.1742x over previous
"""Trainium2 Bass kernel for stacked-Linear dense MLP:
    out[1024, 32768] = x[1024, 512] @ W[32768, 512].T + b[32768]

Baseline v1 (reconstructed): column-parallel over 8 NeuronCores.
"""

import sys

sys.path.insert(0, "/opt/trn_rl_repo")

import numpy as np
import ml_dtypes

# ---- problem constants (hardcoded per contract) ----
B = 1024          # batch (matmul M)
K = 512           # hidden size (contraction)
N_TOTAL = 32768   # hidden_size * map_element_size
N_CORES = 8
NS = N_TOTAL // N_CORES  # 4096 output cols per core

KT = K // 128     # 4 k-tiles
MT = B // 128     # 8 m-tiles
NCH = NS // 512   # 8 n-chunks of 512 (one PSUM bank each)

OUT_BF16 = True   # device writes bf16, host upcasts to fp32

_CACHE = {}


def _build_program():
    import concourse.bacc as bacc
    import concourse.mybir as mybir
    from concourse.bass import ds, ts
    from concourse.tile import TileContext
    from concourse.tile_rust import add_dep_helper
    from contextlib import ExitStack

    nc = bacc.Bacc("TRN2", target_bir_lowering=False, debug=False)

    out_dt = mybir.dt.bfloat16 if OUT_BF16 else mybir.dt.float32

    # host-prepared SBUF-image layouts (see _prep_inputs)
    xh = nc.dram_tensor("xh", [128, MT, KT, 128], mybir.dt.bfloat16, kind="ExternalInput").ap()
    wh = nc.dram_tensor("wh", [128, NCH, KT, 512], mybir.dt.bfloat16, kind="ExternalInput").ap()
    bias = nc.dram_tensor("bias", [1, NS], mybir.dt.float32, kind="ExternalInput").ap()
    out = nc.dram_tensor("out", [B, NS], out_dt, kind="ExternalOutput").ap()

    with TileContext(nc) as tc:
        with ExitStack() as ctx:
            const = ctx.enter_context(tc.tile_pool(name="const", bufs=1))
            outp = ctx.enter_context(tc.tile_pool(name="outp", bufs=12))
            psum = ctx.enter_context(tc.tile_pool(name="psum", bufs=7, space="PSUM"))
            wpool = ctx.enter_context(tc.tile_pool(name="wpool", bufs=1))

            # --- PE warmup ASAP: gpsimd memset (vector is busy with preamble
            # table loads) + warmup matmuls un-throttle HAM before real work.
            # Sized to end right as the first real matmul's inputs land.
            warm = const.tile([128, 512], mybir.dt.bfloat16, tag="warm")
            warm_ps = psum.tile([128, 512], mybir.dt.float32, tag="warmps", bufs=1)
            nc.gpsimd.memset(warm[:], 0)
            for _ in range(10):
                nc.tensor.matmul(
                    warm_ps[:], lhsT=warm[:, 0:128], rhs=warm[:], start=True, stop=True
                )
            warm_sink = const.tile([128, 512], mybir.dt.float32, tag="warmsink")
            nc.vector.tensor_copy(warm_sink[:], warm_ps[:])  # keep warmups live

            # --- x on the scalar ring: two concurrent DMAs sized so each
            # m-tile lands just before the PE's n0 sweep reaches it
            xh_sb = const.tile([128, MT, KT, 128], mybir.dt.bfloat16, tag="xh")
            nc.scalar.dma_start(xh_sb[:, ds(0, 3)], xh[:, ds(0, 3)])
            nc.scalar.dma_start(xh_sb[:, ds(3, 5)], xh[:, ds(3, 5)])

            # --- bias after x on the scalar ring + per-chunk gpsimd broadcasts
            bias_sb = const.tile([128, NS], mybir.dt.float32, tag="bias")
            nc.scalar.dma_start(bias_sb[0:1, :], bias)
            for n in range(NCH):
                nc.gpsimd.partition_broadcast(
                    bias_sb[:, ds(n * 512, 512)], bias_sb[0:1, ds(n * 512, 512)]
                )

            # --- W on the sync ring: chained chunks of [1,2,2,3] n-chunks.
            # First link small (lands first, unblocks the PE); later links
            # big (amortize per-DMA cost, still far ahead of the PE's
            # 6.8us-per-n-chunk consumption rate).
            W_SPLIT = [1, 2, 2, 3]
            wt_tiles = []
            n2cl = {}
            prev = None
            n0 = 0
            for c, sz in enumerate(W_SPLIT):
                t = wpool.tile([128, sz, KT, 512], mybir.dt.bfloat16, tag=f"wt{c}")
                dma = nc.sync.dma_start(t[:], wh[:, ds(n0, sz)])
                if prev is not None:
                    add_dep_helper(dma.ins, prev.ins, reason="chain W DMAs")
                prev = dma
                wt_tiles.append(t)
                for i in range(sz):
                    n2cl[n0 + i] = (c, i)
                n0 += sz

            # --- main loop: n-chunks outer so PE tracks W arrival
            for n in range(NCH):
                for m in range(MT):
                    g = n * MT + m
                    c, ln = n2cl[n]
                    ps = psum.tile([128, 512], mybir.dt.float32)
                    for k in range(KT):
                        nc.tensor.matmul(
                            ps[:],
                            lhsT=xh_sb[:, m, k, :],
                            rhs=wt_tiles[c][:, ln, k, :],
                            start=(k == 0),
                            stop=(k == KT - 1),
                        )
                    ot = outp.tile([128, 512], out_dt)
                    nc.vector.tensor_add(ot[:], ps[:], bias_sb[:, ds(n * 512, 512)])
                    # keep the sync ring clear for the W chain early on
                    if g < 20:
                        eng = nc.scalar
                    else:
                        eng = nc.sync if g % 2 == 0 else nc.scalar
                    eng.dma_start(out[ts(m, 128), ds(n * 512, 512)], ot[:])

    nc.compile()
    return nc


def _get_program():
    if "nc" not in _CACHE:
        _CACHE["nc"] = _build_program()
    return _CACHE["nc"]


def _prep_inputs(x, W, b):
    bf16 = ml_dtypes.bfloat16
    x = np.asarray(x, dtype=np.float32)
    W = np.asarray(W, dtype=np.float32)
    b = np.asarray(b, dtype=np.float32)
    # xh[p, mt, kt, m] = x[mt*128 + m, kt*128 + p]
    xh = np.ascontiguousarray(
        x.T.reshape(KT, 128, MT, 128).transpose(1, 2, 0, 3)
    ).astype(bf16)
    in_maps = []
    for c in range(N_CORES):
        sl = slice(c * NS, (c + 1) * NS)
        # wh[p, n, kt, j] = W[c*NS + n*512 + j, kt*128 + p]
        wh = np.ascontiguousarray(
            W[sl, :].T.reshape(KT, 128, NCH, 512).transpose(1, 2, 0, 3)
        ).astype(bf16)
        bc = np.ascontiguousarray(b[sl].reshape(1, NS))
        in_maps.append({"xh": xh, "wh": wh, "bias": bc})
    return in_maps


def _run(x, W, b, trace=False):
    from concourse.bass_utils import run_bass_kernel_spmd

    nc = _get_program()
    in_maps = _prep_inputs(x, W, b)
    res = run_bass_kernel_spmd(nc, in_maps, list(range(N_CORES)), trace=trace)
    _CACHE["last_result"] = res
    out = np.concatenate([r["out"] for r in res.results], axis=1)
    return out.astype(np.float32)


def kernel(x, W, b):
    return _run(x, W, b, trace=False)


def kernel_profiled(x, W, b):
    """Same as kernel() but with NTFF tracing; returns (out, BassKernelResults)."""
    out = _run(x, W, b, trace=True)
    return out, _CACHE["last_result"]


# revision 14
# speedup vs baseline: 1.2185x; 1.0377x over previous
"""Trainium2 Bass kernel for stacked-Linear dense MLP:
    out[1024, 32768] = x[1024, 512] @ W[32768, 512].T + b[32768]

Baseline v1 (reconstructed): column-parallel over 8 NeuronCores.
"""

import sys

sys.path.insert(0, "/opt/trn_rl_repo")

import numpy as np
import ml_dtypes

# ---- problem constants (hardcoded per contract) ----
B = 1024          # batch (matmul M)
K = 512           # hidden size (contraction)
N_TOTAL = 32768   # hidden_size * map_element_size
N_CORES = 8
NS = N_TOTAL // N_CORES  # 4096 output cols per core

KT = K // 128     # 4 k-tiles
MT = B // 128     # 8 m-tiles
NCH = NS // 512   # 8 n-chunks of 512 (one PSUM bank each)

OUT_BF16 = True   # device writes bf16, host upcasts to fp32

_CACHE = {}


def _build_program():
    import concourse.bacc as bacc
    import concourse.mybir as mybir
    from concourse.bass import ds, ts
    from concourse.tile import TileContext
    from concourse.tile_rust import add_dep_helper
    from contextlib import ExitStack

    nc = bacc.Bacc("TRN2", target_bir_lowering=False, debug=False)

    out_dt = mybir.dt.bfloat16 if OUT_BF16 else mybir.dt.float32

    # host-prepared SBUF-image layouts (see _prep_inputs)
    xh = nc.dram_tensor("xh", [128, MT, KT, 128], mybir.dt.bfloat16, kind="ExternalInput").ap()
    wh = nc.dram_tensor("wh", [128, NCH, KT, 512], mybir.dt.bfloat16, kind="ExternalInput").ap()
    bias = nc.dram_tensor("bias", [128, NS], mybir.dt.bfloat16, kind="ExternalInput").ap()
    out = nc.dram_tensor("out", [B, NS], out_dt, kind="ExternalOutput").ap()

    with TileContext(nc) as tc:
        with ExitStack() as ctx:
            const = ctx.enter_context(tc.tile_pool(name="const", bufs=1))
            outp = ctx.enter_context(tc.tile_pool(name="outp", bufs=18))
            psum = ctx.enter_context(tc.tile_pool(name="psum", bufs=7, space="PSUM"))
            wpool = ctx.enter_context(tc.tile_pool(name="wpool", bufs=1))

            # --- PE warmup ASAP: gpsimd memset (vector is busy with preamble
            # table loads) + warmup matmuls un-throttle HAM before real work.
            # Sized to end right as the first real matmul's inputs land.
            warm = const.tile([128, 512], mybir.dt.bfloat16, tag="warm")
            warm_ps = psum.tile([128, 512], mybir.dt.float32, tag="warmps", bufs=1)
            nc.gpsimd.memset(warm[:], 0)
            for _ in range(11):
                nc.tensor.matmul(
                    warm_ps[:], lhsT=warm[:, 0:128], rhs=warm[:], start=True, stop=True
                )
            # tiny final warmup: the scheduler hoists the first real MM's
            # LDWEIGHTS (with its DMA sem-wait) ahead of the last warmup, so
            # only this one runs after data lands -- keep it cheap
            nc.tensor.matmul(
                warm_ps[:, 0:64], lhsT=warm[:, 0:128], rhs=warm[:, 0:64], start=True, stop=True
            )
            warm_sink = const.tile([128, 512], mybir.dt.float32, tag="warmsink")
            nc.vector.tensor_copy(warm_sink[:], warm_ps[:])  # keep warmups live

            # --- x on the scalar ring: two concurrent DMAs sized so each
            # m-tile lands just before the PE's n0 sweep reaches it
            xh_sb = const.tile([128, MT, KT, 128], mybir.dt.bfloat16, tag="xh")
            nc.scalar.dma_start(xh_sb[:, ds(0, 3)], xh[:, ds(0, 3)])
            nc.scalar.dma_start(xh_sb[:, ds(3, 5)], xh[:, ds(3, 5)])

            # --- bias after x on the scalar ring (host-prebroadcast bf16:
            # no gpsimd broadcasts, no single-partition straggler DMA)
            bias_sb = const.tile([128, NS], mybir.dt.bfloat16, tag="bias")
            nc.scalar.dma_start(bias_sb[:], bias)

            # --- W on the sync ring: chained links [1,1,2,2,2]
            wt_tiles = []
            n2cl = {}
            W_SPLIT = [1, 1, 2, 2, 2]
            prev = None
            n0 = 0
            for c, sz in enumerate(W_SPLIT):
                t = wpool.tile([128, sz, KT, 512], mybir.dt.bfloat16, tag=f"wt{c}", name=f"wt{c}")
                dma = nc.sync.dma_start(t[:], wh[:, ds(n0, sz)])
                if prev is not None:
                    add_dep_helper(dma.ins, prev.ins, reason="chain W DMAs")
                prev = dma
                wt_tiles.append(t)
                for i in range(sz):
                    n2cl[n0 + i] = (c, i)
                n0 += sz

            # --- main loop: n-chunks outer so PE tracks W arrival
            for n in range(NCH):
                for m in range(MT):
                    g = n * MT + m
                    c, ln = n2cl[n]
                    ps = psum.tile([128, 512], mybir.dt.float32)
                    for k in range(KT):
                        nc.tensor.matmul(
                            ps[:],
                            lhsT=xh_sb[:, m, k, :],
                            rhs=wt_tiles[c][:, ln, k, :],
                            start=(k == 0),
                            stop=(k == KT - 1),
                        )
                    ot = outp.tile([128, 512], out_dt)
                    nc.vector.tensor_add(ot[:], ps[:], bias_sb[:, ds(n * 512, 512)])
                    # keep the sync ring clear for the W chain early on
                    if g < 20:
                        eng = nc.scalar
                    elif g == NCH * MT - 1:
                        # last tile: split along the free dim across both
                        # rings (full 128 partitions each) to halve the tail
                        dst = out[ts(m, 128), ds(n * 512, 512)]
                        nc.sync.dma_start(dst[:, 0:256], ot[:, 0:256])
                        nc.scalar.dma_start(dst[:, 256:512], ot[:, 256:512])
                        continue
                    else:
                        eng = nc.sync if g % 2 == 0 else nc.scalar
                    eng.dma_start(out[ts(m, 128), ds(n * 512, 512)], ot[:])
                if n < 2:
                    # boundary absorber: the scheduler prefetches the next
                    # sweep's first LDW (with its W-link sem-wait) one MM
                    # early; this tiny dummy becomes the hostage instead of
                    # delaying this sweep's last real matmul
                    nc.tensor.matmul(
                        warm_ps[:, ds(n * 64, 64)],
                        lhsT=warm[:, 0:128],
                        rhs=warm[:, ds(n * 64, 64)],
                        start=True,
                        stop=True,
                    )
                if n == 2 and m == MT - 1:
                    warm_sink2 = const.tile([128, 128], mybir.dt.float32, tag="warmsink2")
                    nc.vector.tensor_copy(warm_sink2[:], warm_ps[:, 0:128])

    nc.compile()
    return nc


def _get_program():
    if "nc" not in _CACHE:
        _CACHE["nc"] = _build_program()
    return _CACHE["nc"]


def _prep_inputs(x, W, b):
    bf16 = ml_dtypes.bfloat16
    x = np.asarray(x, dtype=np.float32)
    W = np.asarray(W, dtype=np.float32)
    b = np.asarray(b, dtype=np.float32)
    # xh[p, mt, kt, m] = x[mt*128 + m, kt*128 + p]
    xh = np.ascontiguousarray(
        x.T.reshape(KT, 128, MT, 128).transpose(1, 2, 0, 3)
    ).astype(bf16)
    in_maps = []
    for c in range(N_CORES):
        sl = slice(c * NS, (c + 1) * NS)
        # wh[p, n, kt, j] = W[c*NS + n*512 + j, kt*128 + p]
        wh = np.ascontiguousarray(
            W[sl, :].T.reshape(KT, 128, NCH, 512).transpose(1, 2, 0, 3)
        ).astype(bf16)
        bc = np.ascontiguousarray(
            np.broadcast_to(b[sl].reshape(1, NS), (128, NS))
        ).astype(bf16)
        in_maps.append({"xh": xh, "wh": wh, "bias": bc})
    return in_maps


def _run(x, W, b, trace=False):
    from concourse.bass_utils import run_bass_kernel_spmd

    nc = _get_program()
    in_maps = _prep_inputs(x, W, b)
    res = run_bass_kernel_spmd(nc, in_maps, list(range(N_CORES)), trace=trace)
    _CACHE["last_result"] = res
    out = np.concatenate([r["out"] for r in res.results], axis=1)
    return out.astype(np.float32)


def kernel(x, W, b):
    return _run(x, W, b, trace=False)


def kernel_profiled(x, W, b):
    """Same as kernel() but with NTFF tracing; returns (out, BassKernelResults)."""
    out = _run(x, W, b, trace=True)
    return out, _CACHE["last_result"]
